# revision 16
# baseline (speedup 1.0000x reference)
"""LIF spiking-neuron recurrence kernel for Trainium2 (Bass/Tile, 8-core SPMD).

Problem: x [32, 128, 32, 32, 8] f32, time on the LAST axis (T=8).
    u_0 = x_0;  o_t = (u_t > Vth);  u_{t+1} = TAU * u_t * (1 - o_t) + x_{t+1}
Output: spikes o [32, 128, 32, 32, 8] f32 (0.0 / 1.0).

Sharding: pure data-parallel over the batch dim (32 -> 4 per core, 8 cores),
no communication. The host lays each core's shard out t-plane-major
([pixels, T] -> [T, pixels] per 1024-pixel row group) so every on-chip
operand is dense unit-stride. Spikes are exactly 0/1 so the output is
stored as int8, cutting store traffic 4x (per-core HBM traffic 21 MB,
~59 us floor at the ~358 GB/s per-core peak).

With cheap stores, the binding constraint is the Vector engine: fp32
tensor_tensor runs at 1 elem/cycle/lane, and the recurrence nominally needs
three 2-tensor ops per timestep (mask, masked-mult, add) = ~5.8 us per
[128, 2048] plane. This kernel reduces DVE to TWO ops per step by computing
the spike gate on the otherwise-idle Activation engine as a Relu ramp:

    z   = Relu(-BIG*u + BIG*u*)        ACT; u* = nextafter(Vth) so that
                                       z > 0  <=>  u <= Vth (exact for every
                                       f32 input; z >= ~12 whenever nonzero)
    w   = min(TAU*u, z)                DVE scalar_tensor_tensor (mult, min)
                                       == TAU*u*(u<=Vth) exactly, because
                                       TAU*u <= 0.0500000008 < 12 when gated
    o_t = Relu(1 - z) -> int8          ACT; z==0 -> 1, z>=12 -> 0
    u'  = w + x_{t+1}                  DVE tensor_tensor add

TAU*u rounds identically to the reference's TAU*u*(1-o) path, so spike
outputs are bit-exact (including u == Vth exactly, handled by u*).

Stores issue from the idle GPSIMD HWDGE queue and loads from SP, keeping
both off the two busy compute queues; loads are prefetched two steps ahead
instead of all up front so DMA SBUF-write pressure is spread evenly.
"""

import numpy as np

import bass_rust
import concourse.bass as bass
import concourse.mybir as mybir
import concourse.tile as tile
from concourse.bass_utils import run_bass_kernel_spmd

VTH = 0.2
TAU = 0.25

# Gate constants: BIG*(u* - u) with u* = nextafter(f32(0.2)). fma(-BIG, u, B)
# is > 0 exactly when u <= f32(0.2) and otherwise 0 after Relu; the smallest
# positive value it takes is ~12 (one f32 ulp at 0.2 scaled by BIG), safely
# above max(TAU*u) = 0.0500000008, so min(TAU*u, z) never picks z when gated
# on. Exact under both fused and round-between multiply-add.
BIG = 1.0e9
B_GATE = 200000016.0  # float32(BIG * nextafter(float32(0.2)))

N_CORES = 8
FULL_SHAPE = (32, 128, 32, 32, 8)
B_PER_CORE = FULL_SHAPE[0] // N_CORES  # 4
T = FULL_SHAPE[-1]  # 8

ROWS = 256  # per-core partition rows: 4*128*32*32*8 / FREE
FREE = 16384  # free dim per row
C = FREE // T  # 2048 pixels per partition row
N_TILES = ROWS // 128  # 2

_cache: dict = {}


def _split_multi_waits(nc: bass.Bass) -> int:
    """Hoist all-but-one embedded sync waits onto standalone EventSemaphore
    instructions. The walrus build behind bass2jax rejects >1 sync wait per
    instruction ("Too many sync wait commands"); a standalone wait on the
    same engine stream immediately before is semantically identical."""
    n = 0
    for fn in nc.m.functions:
        for block in fn.blocks:
            out = []
            changed = False
            for ins in block.instructions:
                si = ins.sync_info
                waits = list(si.on_wait) if si is not None else []
                if len(waits) > 1:
                    for k, w in enumerate(waits[:-1]):
                        ev = mybir.InstEventSemaphore(
                            name=f"{ins.name}-hw{k}", ins=[], outs=[]
                        )
                        ev.sync_info = bass_rust.SyncInfo(
                            on_wait=[w], on_update=[]
                        )
                        ev.engine = ins.engine
                        nc.inst_map[ev.name] = ev
                        out.append(ev)
                        n += 1
                    si.on_wait = [waits[-1]]
                    changed = True
                out.append(ins)
            if changed:
                block.instructions = out
    return n


def _build_bass() -> bass.Bass:
    f32 = mybir.dt.float32
    i8 = mybir.dt.int8
    Alu = mybir.AluOpType
    Act = mybir.ActivationFunctionType

    nc = bass.Bass(trn_type="TRN2")
    x_d = nc.dram_tensor("x", [ROWS, FREE], f32, kind="ExternalInput")
    y_d = nc.dram_tensor("y", [ROWS, FREE], i8, kind="ExternalOutput")

    # Non-Copy activations take their bias as a per-partition const AP; only
    # 0.0/1.0 are pre-registered, so add the gate bias. Initializing it on
    # the Scalar queue itself (zero, then Copy-with-bias) keeps the init
    # in-order ahead of the first Relu without an all-engine barrier that
    # would hold up the first loads.
    bias_t = nc.alloc_sbuf_tensor("const-bgate", [128, 1], f32)
    nc.scalar.memzero(bias_t.ap())
    nc.scalar.activation(
        bias_t.ap(), bias_t.ap(), mybir.ActivationFunctionType.Copy,
        bias=B_GATE, scale=1.0,
    )
    nc.const_aps.aps[(f32, B_GATE)] = bias_t.ap()

    with tile.TileContext(nc) as tc:
        with (
            tc.tile_pool(name="pin", bufs=4) as pin,
            tc.tile_pool(name="pout", bufs=6) as pout,
            tc.tile_pool(name="pz", bufs=5) as pz,
            tc.tile_pool(name="pu", bufs=6) as pu,
        ):
            row_sl = [slice(i * 128, (i + 1) * 128) for i in range(N_TILES)]

            # initial membrane state u_0 = x_0: plain HWDGE loads on SP
            u = []
            for i in range(N_TILES):
                p = pin.tile([128, C], f32, tag="xp")
                nc.sync.dma_start(p, x_d[row_sl[i], 0:C])
                u.append(p)

            for t in range(T - 1):
                for i in range(N_TILES):
                    cols = slice(t * C, (t + 1) * C)
                    # ACT: spike gate z = Relu(BIG*(u* - u))
                    z = pz.tile([128, C], f32, tag="z")
                    nc.scalar.activation(
                        z, u[i], Act.Relu, bias=B_GATE, scale=-BIG
                    )
                    # spike output, alternating between DVE (straight
                    # compare) and ACT (from z) to balance the two queues
                    o_t = pout.tile([128, C], i8, tag="o")
                    if (t + i) % 2 == 0:
                        nc.vector.tensor_scalar(
                            o_t, u[i], VTH, None, Alu.is_gt
                        )
                    else:
                        nc.scalar.activation(
                            o_t, z, Act.Relu, bias=1.0, scale=-1.0
                        )
                    # DVE: w = min(TAU*u, z) == TAU*u*(u <= Vth)
                    w = pu.tile([128, C], f32, tag="u")
                    nc.vector.scalar_tensor_tensor(
                        w, u[i], TAU, z, Alu.mult, Alu.min
                    )
                    # u' = w + x_{t+1}: the add happens INSIDE the load -
                    # SWDGE accumulate-DMA sums the incoming plane into w,
                    # so the state update costs no engine time at all
                    nc.gpsimd.dma_start(
                        w,
                        x_d[row_sl[i], (t + 1) * C : (t + 2) * C],
                        accum_op=Alu.add,
                    )
                    # store via SP's HWDGE queue (it only carries the two
                    # initial loads, and HWDGE avoids the SWDGE rings)
                    nc.sync.dma_start(y_d[row_sl[i], cols], o_t)
                    u[i] = w

                    if t == T - 2:
                        # this tile's last-step compare, right behind its
                        # final accumulate so the kernel tail stays short
                        o_l = pout.tile([128, C], i8, tag="o")
                        nc.vector.tensor_scalar(
                            o_l, w, VTH, None, Alu.is_gt
                        )
                        nc.sync.dma_start(
                            y_d[row_sl[i], (T - 1) * C : T * C], o_l
                        )

    _split_multi_waits(nc)
    return nc


def _shard(x: np.ndarray, c: int) -> np.ndarray:
    """Core c's shard, t-plane-major: [ROWS, C, T] -> [ROWS, T, C] -> flat."""
    s = x[c * B_PER_CORE : (c + 1) * B_PER_CORE].reshape(ROWS, C, T)
    return np.ascontiguousarray(s.transpose(0, 2, 1)).reshape(ROWS, FREE)


def _unshard(y: np.ndarray) -> np.ndarray:
    """Invert _shard's layout for one core's int8 0/1 output -> f32."""
    o = (y > 0).astype(np.float32)
    s = o.reshape(ROWS, T, C).transpose(0, 2, 1)
    return np.ascontiguousarray(s).reshape(B_PER_CORE, *FULL_SHAPE[1:])


def kernel(x: np.ndarray) -> np.ndarray:
    assert x.shape == FULL_SHAPE, x.shape
    in_dtype = x.dtype

    if "nc" not in _cache:
        _cache["nc"] = _build_bass()
    nc = _cache["nc"]

    x = np.ascontiguousarray(x, dtype=np.float32)
    in_maps = [{"x": _shard(x, c)} for c in range(N_CORES)]
    res = run_bass_kernel_spmd(nc, in_maps, core_ids=list(range(N_CORES)))
    out = np.concatenate(
        [_unshard(res.results[c]["y"]) for c in range(N_CORES)], axis=0
    )
    return out.astype(in_dtype, copy=False)


# revision 18
# speedup vs baseline: 1.2770x; 1.2770x over previous
"""LIF spiking-neuron recurrence kernel for Trainium2 (Bass/Tile, 8-core SPMD).

Problem: x [32, 128, 32, 32, 8] f32, time on the LAST axis (T=8).
    u_0 = x_0;  o_t = (u_t > Vth);  u_{t+1} = TAU * u_t * (1 - o_t) + x_{t+1}
Output: spikes o [32, 128, 32, 32, 8] f32 (0.0 / 1.0).

Sharding: pure data-parallel over the batch dim (32 -> 4 per core, 8 cores),
no communication. The host lays each core's shard out t-plane-major
([pixels, T] -> [T, pixels] per 1024-pixel row group) so every on-chip
operand is dense unit-stride. Spikes are exactly 0/1 so the output is
stored as int8, cutting store traffic 4x (per-core HBM traffic 21 MB,
~59 us floor at the ~358 GB/s per-core peak).

With cheap stores, the binding constraint is the Vector engine: fp32
tensor_tensor runs at 1 elem/cycle/lane, and the recurrence nominally needs
three 2-tensor ops per timestep (mask, masked-mult, add) = ~5.8 us per
[128, 2048] plane. This kernel reduces DVE to TWO ops per step by computing
the spike gate on the otherwise-idle Activation engine as a Relu ramp:

    z   = Relu(-BIG*u + BIG*u*)        ACT; u* = nextafter(Vth) so that
                                       z > 0  <=>  u <= Vth (exact for every
                                       f32 input; z >= ~12 whenever nonzero)
    w   = min(TAU*u, z)                DVE scalar_tensor_tensor (mult, min)
                                       == TAU*u*(u<=Vth) exactly, because
                                       TAU*u <= 0.0500000008 < 12 when gated
    o_t = Relu(1 - z) -> int8          ACT; z==0 -> 1, z>=12 -> 0
    u'  = w + x_{t+1}                  DVE tensor_tensor add

TAU*u rounds identically to the reference's TAU*u*(1-o) path, so spike
outputs are bit-exact (including u == Vth exactly, handled by u*).

Stores issue from the idle GPSIMD HWDGE queue and loads from SP, keeping
both off the two busy compute queues; loads are prefetched two steps ahead
instead of all up front so DMA SBUF-write pressure is spread evenly.
"""

import numpy as np

import bass_rust
import concourse.bass as bass
import concourse.mybir as mybir
import concourse.tile as tile
from concourse.bass_utils import run_bass_kernel_spmd

VTH = 0.2
TAU = 0.25

# Gate constants: BIG*(u* - u) with u* = nextafter(f32(0.2)). fma(-BIG, u, B)
# is > 0 exactly when u <= f32(0.2) and otherwise 0 after Relu; the smallest
# positive value it takes is ~12 (one f32 ulp at 0.2 scaled by BIG), safely
# above max(TAU*u) = 0.0500000008, so min(TAU*u, z) never picks z when gated
# on. Exact under both fused and round-between multiply-add.
BIG = 1.0e9
B_GATE = 200000016.0  # float32(BIG * nextafter(float32(0.2)))

N_CORES = 8
FULL_SHAPE = (32, 128, 32, 32, 8)
B_PER_CORE = FULL_SHAPE[0] // N_CORES  # 4
T = FULL_SHAPE[-1]  # 8

ROWS = 256  # per-core partition rows: 4*128*32*32*8 / FREE
FREE = 16384  # free dim per row
C = FREE // T  # 2048 pixels per partition row
N_TILES = ROWS // 128  # 2

_cache: dict = {}


def _split_multi_waits(nc: bass.Bass) -> int:
    """Hoist all-but-one embedded sync waits onto standalone EventSemaphore
    instructions. The walrus build behind bass2jax rejects >1 sync wait per
    instruction ("Too many sync wait commands"); a standalone wait on the
    same engine stream immediately before is semantically identical."""
    n = 0
    for fn in nc.m.functions:
        for block in fn.blocks:
            out = []
            changed = False
            for ins in block.instructions:
                si = ins.sync_info
                waits = list(si.on_wait) if si is not None else []
                if len(waits) > 1:
                    for k, w in enumerate(waits[:-1]):
                        ev = mybir.InstEventSemaphore(
                            name=f"{ins.name}-hw{k}", ins=[], outs=[]
                        )
                        ev.sync_info = bass_rust.SyncInfo(
                            on_wait=[w], on_update=[]
                        )
                        ev.engine = ins.engine
                        nc.inst_map[ev.name] = ev
                        out.append(ev)
                        n += 1
                    si.on_wait = [waits[-1]]
                    changed = True
                out.append(ins)
            if changed:
                block.instructions = out
    return n


def _build_bass() -> bass.Bass:
    f32 = mybir.dt.float32
    i8 = mybir.dt.int8
    Alu = mybir.AluOpType
    Act = mybir.ActivationFunctionType

    nc = bass.Bass(trn_type="TRN2")
    x_d = nc.dram_tensor("x", [ROWS, FREE], f32, kind="ExternalInput")
    y_d = nc.dram_tensor("y", [ROWS, FREE], i8, kind="ExternalOutput")

    # Non-Copy activations take their bias as a per-partition const AP; only
    # 0.0/1.0 are pre-registered, so add the gate bias. Initializing it on
    # the Scalar queue itself (zero, then Copy-with-bias) keeps the init
    # in-order ahead of the first Relu without an all-engine barrier that
    # would hold up the first loads.
    bias_t = nc.alloc_sbuf_tensor("const-bgate", [128, 1], f32)
    nc.scalar.memzero(bias_t.ap())
    nc.scalar.activation(
        bias_t.ap(), bias_t.ap(), mybir.ActivationFunctionType.Copy,
        bias=B_GATE, scale=1.0,
    )
    nc.const_aps.aps[(f32, B_GATE)] = bias_t.ap()

    with tile.TileContext(nc) as tc:
        with (
            tc.tile_pool(name="pin", bufs=7) as pin,
            tc.tile_pool(name="pout", bufs=6) as pout,
            tc.tile_pool(name="pz", bufs=5) as pz,
            tc.tile_pool(name="pu", bufs=6) as pu,
        ):
            row_sl = [slice(i * 128, (i + 1) * 128) for i in range(N_TILES)]

            # Columns [0, D) of each state-update add run on DVE (in-place
            # tensor_tensor into w); columns [D, C) ride along inside the
            # load itself as an SWDGE accumulate-DMA (~155 GB/s effective,
            # ~2.3x plain-DMA cost, paid out of spare DMA bandwidth).
            D = C - 256

            # initial membrane state u_0 = x_0: plain HWDGE loads on SP
            u = []
            for i in range(N_TILES):
                p = pu.tile([128, C], f32, tag="u")
                nc.sync.dma_start(p, x_d[row_sl[i], 0:C])
                u.append(p)

            # plain part of later planes, prefetched two steps ahead
            def load(i, t):
                p = pin.tile([128, D], f32, tag="xp")
                nc.sync.dma_start(p, x_d[row_sl[i], t * C : t * C + D])
                return p

            xp = [[None] * T for _ in range(N_TILES)]
            for t in range(1, 3):
                for i in range(N_TILES):
                    xp[i][t] = load(i, t)

            for t in range(T - 1):
                for i in range(N_TILES):
                    cols = slice(t * C, (t + 1) * C)
                    if t + 3 < T:
                        xp[i][t + 3] = load(i, t + 3)
                    # ACT: spike gate z = Relu(BIG*(u* - u))
                    z = pz.tile([128, C], f32, tag="z")
                    nc.scalar.activation(
                        z, u[i], Act.Relu, bias=B_GATE, scale=-BIG
                    )
                    # ACT: o = Relu(1 - z) in {0, 1} -> int8
                    o_t = pout.tile([128, C], i8, tag="o")
                    nc.scalar.activation(
                        o_t, z, Act.Relu, bias=1.0, scale=-1.0
                    )
                    # DVE: w = min(TAU*u, z) == TAU*u*(u <= Vth)
                    w = pu.tile([128, C], f32, tag="u")
                    nc.vector.scalar_tensor_tensor(
                        w, u[i], TAU, z, Alu.mult, Alu.min
                    )
                    # u' = w + x_{t+1}: DVE adds the prefetched plain part
                    # in place; the accumulate-DMA sums the tail columns of
                    # the incoming plane straight into w
                    nc.vector.tensor_tensor(
                        w[:, :D], w[:, :D], xp[i][t + 1], Alu.add
                    )
                    nc.gpsimd.dma_start(
                        w[:, D:],
                        x_d[row_sl[i], (t + 1) * C + D : (t + 2) * C],
                        accum_op=Alu.add,
                    )
                    # store via SP's HWDGE queue (HWDGE avoids SWDGE rings)
                    nc.sync.dma_start(y_d[row_sl[i], cols], o_t)
                    u[i] = w

                    if t == T - 2:
                        # this tile's last-step compare, right behind its
                        # final accumulate so the kernel tail stays short
                        o_l = pout.tile([128, C], i8, tag="o")
                        nc.vector.tensor_scalar(
                            o_l, w, VTH, None, Alu.is_gt
                        )
                        nc.sync.dma_start(
                            y_d[row_sl[i], (T - 1) * C : T * C], o_l
                        )

    _split_multi_waits(nc)
    return nc


def _shard(x: np.ndarray, c: int) -> np.ndarray:
    """Core c's shard, t-plane-major: [ROWS, C, T] -> [ROWS, T, C] -> flat."""
    s = x[c * B_PER_CORE : (c + 1) * B_PER_CORE].reshape(ROWS, C, T)
    return np.ascontiguousarray(s.transpose(0, 2, 1)).reshape(ROWS, FREE)


def _unshard(y: np.ndarray) -> np.ndarray:
    """Invert _shard's layout for one core's int8 0/1 output -> f32."""
    o = (y > 0).astype(np.float32)
    s = o.reshape(ROWS, T, C).transpose(0, 2, 1)
    return np.ascontiguousarray(s).reshape(B_PER_CORE, *FULL_SHAPE[1:])


def kernel(x: np.ndarray) -> np.ndarray:
    assert x.shape == FULL_SHAPE, x.shape
    in_dtype = x.dtype

    if "nc" not in _cache:
        _cache["nc"] = _build_bass()
    nc = _cache["nc"]

    x = np.ascontiguousarray(x, dtype=np.float32)
    in_maps = [{"x": _shard(x, c)} for c in range(N_CORES)]
    res = run_bass_kernel_spmd(nc, in_maps, core_ids=list(range(N_CORES)))
    out = np.concatenate(
        [_unshard(res.results[c]["y"]) for c in range(N_CORES)], axis=0
    )
    return out.astype(in_dtype, copy=False)


# revision 23
# speedup vs baseline: 1.4657x; 1.1477x over previous
"""LIF spiking-neuron recurrence kernel for Trainium2 (Bass/Tile, 8-core SPMD).

Problem: x [32, 128, 32, 32, 8] f32, time on the LAST axis (T=8).
    u_0 = x_0;  o_t = (u_t > Vth);  u_{t+1} = TAU * u_t * (1 - o_t) + x_{t+1}
Output: spikes o [32, 128, 32, 32, 8] f32 (0.0 / 1.0).

Sharding: pure data-parallel over the batch dim (32 -> 4 per core, 8 cores),
no communication. The host lays each core's shard out t-plane-major
([pixels, T] -> [T, pixels] per 1024-pixel row group) so every on-chip
operand is dense unit-stride. Spikes are exactly 0/1 so the output is
stored as int8, cutting store traffic 4x (per-core HBM traffic 21 MB,
~59 us floor at the ~358 GB/s per-core peak).

With cheap stores, the binding constraint is the Vector engine: fp32
tensor_tensor runs at 1 elem/cycle/lane, and the recurrence nominally needs
three 2-tensor ops per timestep (mask, masked-mult, add) = ~5.8 us per
[128, 2048] plane. This kernel reduces DVE to TWO ops per step by computing
the spike gate on the otherwise-idle Activation engine as a Relu ramp:

    z   = Relu(-BIG*u + BIG*u*)        ACT; u* = nextafter(Vth) so that
                                       z > 0  <=>  u <= Vth (exact for every
                                       f32 input; z >= ~12 whenever nonzero)
    w   = min(TAU*u, z)                DVE scalar_tensor_tensor (mult, min)
                                       == TAU*u*(u<=Vth) exactly, because
                                       TAU*u <= 0.0500000008 < 12 when gated
    o_t = Relu(1 - z) -> int8          ACT; z==0 -> 1, z>=12 -> 0
    u'  = w + x_{t+1}                  DVE tensor_tensor add

TAU*u rounds identically to the reference's TAU*u*(1-o) path, so spike
outputs are bit-exact (including u == Vth exactly, handled by u*).

Stores issue from the idle GPSIMD HWDGE queue and loads from SP, keeping
both off the two busy compute queues; loads are prefetched two steps ahead
instead of all up front so DMA SBUF-write pressure is spread evenly.
"""

import numpy as np

import bass_rust
import concourse.bass as bass
import concourse.mybir as mybir
import concourse.tile as tile
from concourse.bass_utils import run_bass_kernel_spmd

VTH = 0.2
TAU = 0.25

# Gate constants: BIG*(u* - u) with u* = nextafter(f32(0.2)). fma(-BIG, u, B)
# is > 0 exactly when u <= f32(0.2) and otherwise 0 after Relu; the smallest
# positive value it takes is ~12 (one f32 ulp at 0.2 scaled by BIG), safely
# above max(TAU*u) = 0.0500000008, so min(TAU*u, z) never picks z when gated
# on. Exact under both fused and round-between multiply-add.
BIG = 1.0e9
B_GATE = 200000016.0  # float32(BIG * nextafter(float32(0.2)))

N_CORES = 8
FULL_SHAPE = (32, 128, 32, 32, 8)
B_PER_CORE = FULL_SHAPE[0] // N_CORES  # 4
T = FULL_SHAPE[-1]  # 8

ROWS = 256  # per-core partition rows: 4*128*32*32*8 / FREE
FREE = 16384  # free dim per row
C = FREE // T  # 2048 pixels per partition row
N_TILES = ROWS // 128  # 2

_cache: dict = {}


def _split_multi_waits(nc: bass.Bass) -> int:
    """Hoist all-but-one embedded sync waits onto standalone EventSemaphore
    instructions. The walrus build behind bass2jax rejects >1 sync wait per
    instruction ("Too many sync wait commands"); a standalone wait on the
    same engine stream immediately before is semantically identical."""
    n = 0
    for fn in nc.m.functions:
        for block in fn.blocks:
            out = []
            changed = False
            for ins in block.instructions:
                si = ins.sync_info
                waits = list(si.on_wait) if si is not None else []
                if len(waits) > 1:
                    for k, w in enumerate(waits[:-1]):
                        ev = mybir.InstEventSemaphore(
                            name=f"{ins.name}-hw{k}", ins=[], outs=[]
                        )
                        ev.sync_info = bass_rust.SyncInfo(
                            on_wait=[w], on_update=[]
                        )
                        ev.engine = ins.engine
                        nc.inst_map[ev.name] = ev
                        out.append(ev)
                        n += 1
                    si.on_wait = [waits[-1]]
                    changed = True
                out.append(ins)
            if changed:
                block.instructions = out
    return n


def _build_bass() -> bass.Bass:
    f32 = mybir.dt.float32
    i8 = mybir.dt.int8
    Alu = mybir.AluOpType
    Act = mybir.ActivationFunctionType

    nc = bass.Bass(trn_type="TRN2")
    x_d = nc.dram_tensor("x", [ROWS, FREE], f32, kind="ExternalInput")
    y_d = nc.dram_tensor("y", [ROWS, FREE], i8, kind="ExternalOutput")

    # Non-Copy activations take their bias as a per-partition const AP; only
    # 0.0/1.0 are pre-registered, so add the gate bias. Initializing it on
    # the Scalar queue itself (zero, then Copy-with-bias) keeps the init
    # in-order ahead of the first Relu without an all-engine barrier that
    # would hold up the first loads.
    bias_t = nc.alloc_sbuf_tensor("const-bgate", [128, 1], f32)
    nc.scalar.memzero(bias_t.ap())
    nc.scalar.activation(
        bias_t.ap(), bias_t.ap(), mybir.ActivationFunctionType.Copy,
        bias=B_GATE, scale=1.0,
    )
    nc.const_aps.aps[(f32, B_GATE)] = bias_t.ap()

    with tile.TileContext(nc) as tc:
        with (
            tc.tile_pool(name="pin", bufs=8) as pin,
            tc.tile_pool(name="pout", bufs=6) as pout,
            tc.tile_pool(name="pz", bufs=4) as pz,
            tc.tile_pool(name="pw", bufs=4) as pw,
            tc.tile_pool(name="pu", bufs=4) as pu,
        ):
            row_sl = [slice(i * 128, (i + 1) * 128) for i in range(N_TILES)]

            def load(i, t):
                p = pin.tile([128, C], f32, tag="xp")
                nc.sync.dma_start(p, x_d[row_sl[i], t * C : (t + 1) * C])
                return p

            # Plane 0 arrives as four quarter-tiles per row tile so the
            # first gate starts after 256 KiB instead of a full 1 MiB
            # (dependencies are tracked per tile, so the split must happen
            # at the load itself).
            NQ = 4
            S0 = C // NQ
            xq = [[None] * NQ for _ in range(N_TILES)]
            for i in range(N_TILES):
                for q in range(NQ):
                    p = pin.tile([128, S0], f32, tag="xq")
                    nc.sync.dma_start(
                        p, x_d[row_sl[i], q * S0 : (q + 1) * S0]
                    )
                    xq[i][q] = p

            # three-step prefetch for the rest: planes 1-2 of both tiles
            # land next, the rest stream in two steps ahead of their add
            PF = 3
            xp = [[None] * T for _ in range(N_TILES)]
            for t in range(1, PF):
                for i in range(N_TILES):
                    xp[i][t] = load(i, t)

            u = [None] * N_TILES
            for t in range(T - 1):
                # Step 0 runs in quarter-planes: its z can start as soon as
                # the first quarter of plane 0 lands instead of waiting for
                # the whole 1 MiB transfer, pulling the pipeline earlier.
                # Later steps use full planes (lower per-op overhead).
                n_chunks = NQ if t == 0 else 1
                S = C // n_chunks
                for i in range(N_TILES):
                    if t + PF < T:
                        xp[i][t + PF] = load(i, t + PF)

                    o_t = pout.tile([128, C], i8, tag="o")
                    w = pw.tile([128, C], f32, tag="w")
                    un = pu.tile([128, C], f32, tag="u")
                    for q in range(n_chunks):
                        qs = slice(q * S, (q + 1) * S)
                        u_src = xq[i][q] if t == 0 else u[i][:, qs]
                        # ACT: spike gate z = Relu(BIG*(u* - u))
                        z = pz.tile([128, S], f32, tag=f"z{t == 0}")
                        nc.scalar.activation(
                            z, u_src, Act.Relu, bias=B_GATE, scale=-BIG
                        )
                        # ACT: o = Relu(1 - z) in {0, 1} -> int8
                        nc.scalar.activation(
                            o_t[:, qs], z, Act.Relu, bias=1.0, scale=-1.0
                        )
                        # DVE: w = min(TAU*u, z) == TAU*u*(u <= Vth)
                        nc.vector.scalar_tensor_tensor(
                            w[:, qs], u_src, TAU, z, Alu.mult, Alu.min
                        )
                        # DVE: u' = w + x_{t+1}
                        nc.vector.tensor_tensor(
                            un[:, qs], w[:, qs], xp[i][t + 1][:, qs], Alu.add
                        )
                    # store from the otherwise-idle GPSIMD HWDGE queue
                    nc.gpsimd.dma_start(
                        y_d[row_sl[i], t * C : (t + 1) * C], o_t
                    )
                    u[i] = un

                    if t == T - 2:
                        # Emit this tile's last-step compare right behind its
                        # final add so the kernel tail is one op, not two.
                        # It runs on DVE: GPSIMD's tensor_scalar measures
                        # ~32us/plane (25x DVE) and starves DVE via the
                        # shared SBUF port, so GPSIMD only ever issues DMA.
                        o_l = pout.tile([128, C], i8, tag="o")
                        nc.vector.tensor_scalar(
                            o_l, un, VTH, None, Alu.is_gt
                        )
                        nc.gpsimd.dma_start(
                            y_d[row_sl[i], (T - 1) * C : T * C], o_l
                        )

    _split_multi_waits(nc)
    return nc


def _shard(x: np.ndarray, c: int) -> np.ndarray:
    """Core c's shard, t-plane-major: [ROWS, C, T] -> [ROWS, T, C] -> flat."""
    s = x[c * B_PER_CORE : (c + 1) * B_PER_CORE].reshape(ROWS, C, T)
    return np.ascontiguousarray(s.transpose(0, 2, 1)).reshape(ROWS, FREE)


def _unshard(y: np.ndarray) -> np.ndarray:
    """Invert _shard's layout for one core's int8 0/1 output -> f32."""
    o = (y > 0).astype(np.float32)
    s = o.reshape(ROWS, T, C).transpose(0, 2, 1)
    return np.ascontiguousarray(s).reshape(B_PER_CORE, *FULL_SHAPE[1:])


def kernel(x: np.ndarray) -> np.ndarray:
    assert x.shape == FULL_SHAPE, x.shape
    in_dtype = x.dtype

    if "nc" not in _cache:
        _cache["nc"] = _build_bass()
    nc = _cache["nc"]

    x = np.ascontiguousarray(x, dtype=np.float32)
    in_maps = [{"x": _shard(x, c)} for c in range(N_CORES)]
    res = run_bass_kernel_spmd(nc, in_maps, core_ids=list(range(N_CORES)))
    out = np.concatenate(
        [_unshard(res.results[c]["y"]) for c in range(N_CORES)], axis=0
    )
    return out.astype(in_dtype, copy=False)
